# revision 2
# baseline (speedup 1.0000x reference)
"""Trainium2 Bass kernel v2 for nn_MultiHeadAttention_60971355734022.

Full inputs in, full output out. Sharding: 8 cores = 4 batches x 2 head-groups
(8 heads each); host combines out[b] = core(2b) + core(2b+1) + bf.

v2 restructure vs v1 (422us): aimed at continuous PE occupancy (p-state) and
hiding the ACT-bound softmax exp under PE work.
  - inputs pre-transposed on host ([128, sc, db, 512] fp16) - no xbar DMAs
  - one-time PE warmup chain (ramps the tensor engine while inputs DMA)
  - attention runs in 512-wide q-spans: scores psum [128, 2, 512] (2 banks,
    double buffered), PV accum [65, 2, 512] (2 banks), leaving 2 banks for a
    shared aux pool used by projections / v-proj / output proj
  - emission order interleaves the q/k projections for head-pair hp+1, the
    v-projection k-blocks, and the output projection into the exp-bound
    attention loop of head-pair hp, so the PE never drains
  - exp processes both heads of a pair in one ACT instruction (3D AP)
"""
import sys

sys.path.insert(0, "/opt/trn_rl_repo")

import math

import numpy as np

import concourse.bacc as bacc
import concourse.bass as bass
import concourse.tile as tile
from concourse import mybir
from concourse.bass_utils import run_bass_kernel_spmd

F32 = mybir.dt.float32
F16 = mybir.dt.float16
F8 = mybir.dt.float8e4
DR = mybir.MatmulPerfMode.DoubleRow

S = 2048          # sequence length per batch
D = 1024          # model dim
P = 512           # per-core projection cols (8 heads x 64)
NH = 8            # heads per core
DH = 64           # head dim
NKB = S // 128    # 16 k-blocks
QS = 512          # q-span for attention phase
NSP = S // QS     # 4 q-spans
SCALE = 1.0 / math.sqrt(2048.0)  # reference scales by 1/sqrt(MAX_LEN)

EXP = mybir.ActivationFunctionType.Exp


QK_FP8 = True   # q/k projections via fp8e4 DoubleRow (empirical gate 6.4e-3)
QK8_SCORES = False  # qhT/khT stored fp8, scores via DoubleRow (gate 7.3e-3)
FQK = F8 if QK_FP8 else F16


def build_core_kernel(repeat=1):
    nc = bacc.Bacc()

    xq = nc.dram_tensor("xq", [128, NSP, 8, QS], FQK, kind="ExternalInput")
    xk = nc.dram_tensor("xk", [128, NSP, 8, QS], FQK, kind="ExternalInput")
    xv = nc.dram_tensor("xv", [128, NSP, 8, QS], F16, kind="ExternalInput")
    wq = nc.dram_tensor("wq", [128, 8, P], FQK, kind="ExternalInput")
    wk = nc.dram_tensor("wk", [128, 8, P], FQK, kind="ExternalInput")
    wv = nc.dram_tensor("wv", [128, 8, P], F16, kind="ExternalInput")
    wf = nc.dram_tensor("wf", [128, 4, D], F16, kind="ExternalInput")
    bqv = nc.dram_tensor("bqv", [128, 4], F32, kind="ExternalInput")
    bkv = nc.dram_tensor("bkv", [128, 4], F32, kind="ExternalInput")
    bvv = nc.dram_tensor("bvv", [1, P], F16, kind="ExternalInput")
    vones = nc.dram_tensor("vones", [128, NKB, NH, 1], F16, kind="ExternalInput")
    out = nc.dram_tensor("out", [S, D], F32, kind="ExternalOutput")

    with tile.TileContext(nc) as tc:
        with tc.tile_pool(name="persist", bufs=1) as pp:
            xtq = pp.tile([128, NSP, 8, QS], FQK, name="xtq", tag="xtq")
            xtk = pp.tile([128, NSP, 8, QS], FQK, name="xtk", tag="xtk")
            wtq = pp.tile([128, 8, P], FQK, name="wtq", tag="wtq")
            wtk = pp.tile([128, 8, P], FQK, name="wtk", tag="wtk")
            wtv = pp.tile([128, 8, P], F16, name="wtv", tag="wtv")
            wft = pp.tile([128, 4, D], F16, name="wft", tag="wft")
            bq_sb = pp.tile([128, 4], F32, name="bq_sb", tag="bq_sb")
            bk_sb = pp.tile([128, 4], F32, name="bk_sb", tag="bk_sb")
            bv_bc = pp.tile([128, P], F16, name="bv_bc", tag="bv_bc")
            if QK8_SCORES:
                # quad layout: tile t holds heads 4t..4t+3; head a on
                # partitions 32a..32a+32, dh split across the 2 DR slots
                qhT = [pp.tile([128, 2, S], F8, name=f"qhT{i}", tag=f"qhT{i}")
                       for i in range(2)]
                khT = [pp.tile([128, 2, S], F8, name=f"khT{i}", tag=f"khT{i}")
                       for i in range(2)]
            else:
                qhT = [pp.tile([128, S], F16, name=f"qhT{i}", tag=f"qhT{i}")
                       for i in range(4)]
                khT = [pp.tile([128, S], F16, name=f"khT{i}", tag=f"khT{i}")
                       for i in range(4)]
            vhh = pp.tile([128, NKB, NH, DH + 1], F16, name="vhh", tag="vhh")
            cT = [pp.tile([128, S], F16, name=f"cT{i}", tag=f"cT{i}")
                  for i in range(4)]

            # ---- prelude DMAs: weights (gpsimd queue), inputs (sync queue)
            nc.gpsimd.dma_start(out=wtq, in_=wq[:, :, :])
            nc.gpsimd.dma_start(out=wtk, in_=wk[:, :, :])
            nc.gpsimd.dma_start(out=wtv, in_=wv[:, :, :])
            nc.gpsimd.dma_start(out=wft, in_=wf[:, :, :])
            nc.gpsimd.dma_start(out=bq_sb, in_=bqv[:, :])
            nc.gpsimd.dma_start(out=bk_sb, in_=bkv[:, :])
            bv_row = pp.tile([1, P], F16, name="bv_row", tag="bv_row")
            nc.gpsimd.dma_start(out=bv_row, in_=bvv[:, :])
            nc.gpsimd.partition_broadcast(bv_bc, bv_row)
            nc.sync.dma_start(out=vhh[:, :, :, DH:DH + 1], in_=vones[:, :, :, :])
            for sc in range(NSP):
                nc.sync.dma_start(out=xtq[:, sc], in_=xq[:, sc])
            for sc in range(NSP):
                nc.sync.dma_start(out=xtk[:, sc], in_=xk[:, sc])

            # ---- one-time PE warmup: ramp p-state while inputs stream in
            with tc.tile_pool(name="wup", bufs=2, space="PSUM") as wup:
                for i in range(16):
                    wt = wup.tile([128, P], F32, name=f"wu{i}", tag="wu")
                    nc.tensor.matmul(wt[:, :], wtv[:, i % 8, 0:128],
                                     wtv[:, i % 8, :], start=True, stop=True)

            def _phases():
                with tc.tile_pool(name="aux", bufs=2, space="PSUM") as auxp, \
                     tc.tile_pool(name="sps", bufs=2, space="PSUM") as spsp, \
                     tc.tile_pool(name="ops", bufs=1, space="PSUM") as opsp, \
                     tc.tile_pool(name="ptp", bufs=4) as ptp, \
                     tc.tile_pool(name="nrm", bufs=2) as nrmp, \
                     tc.tile_pool(name="osg", bufs=3) as osgp, \
                     tc.tile_pool(name="xvp", bufs=4) as xvp:

                    def proj_qk_sc(which, pb, sc):
                        wt = wtq if which == "q" else wtk
                        dst = qhT if which == "q" else khT
                        bias = bq_sb if which == "q" else bk_sb
                        xt = xtq if which == "q" else xtk
                        pj = auxp.tile([128, QS], F32,
                                       name=f"pj_{which}{pb}{sc}", tag="aux")
                        if QK_FP8:
                            for j in range(4):
                                nc.tensor.matmul(
                                    pj[:, :],
                                    wt[:, 2 * j:2 * j + 2,
                                       128 * pb:128 * pb + 128],
                                    xt[:, sc, 2 * j:2 * j + 2, :],
                                    start=(j == 0), stop=(j == 3),
                                    perf_mode=DR)
                        else:
                            for db in range(8):
                                nc.tensor.matmul(
                                    pj[:, :],
                                    wt[:, db, 128 * pb:128 * pb + 128],
                                    xt[:, sc, db, :],
                                    start=(db == 0), stop=(db == 7))
                        if QK8_SCORES:
                            nc.vector.tensor_scalar_add(
                                dst[pb // 2][:, pb % 2, QS * sc:QS * (sc + 1)],
                                pj[:, :], bias[:, pb:pb + 1])
                        else:
                            nc.vector.tensor_scalar_add(
                                dst[pb][:, QS * sc:QS * (sc + 1)],
                                pj[:, :], bias[:, pb:pb + 1])

                    def proj_qk(which, pb):
                        for sc in range(NSP):
                            proj_qk_sc(which, pb, sc)

                    def proj_v(sg):
                        xvt = xvp.tile([128, 8, 128], F16,
                                       name=f"xvt{sg}", tag="xvt")
                        nc.sync.dma_start(
                            out=xvt,
                            in_=xv[:, sg // 4, :,
                                   128 * (sg % 4):128 * (sg % 4) + 128])
                        pj = auxp.tile([128, P], F32, name=f"pj_v{sg}", tag="aux")
                        for db in range(8):
                            nc.tensor.matmul(
                                pj[:, :],
                                xvt[:, db, :],
                                wtv[:, db, :],
                                start=(db == 0), stop=(db == 7))
                        nc.vector.scalar_tensor_tensor(
                            vhh[:, sg, :, 0:DH],
                            pj.rearrange("p (h d) -> p h d", h=NH),
                            1.0,
                            bv_bc.rearrange("p (h d) -> p h d", h=NH),
                            mybir.AluOpType.mult,
                            mybir.AluOpType.add)

                    def attn_score(hp, ps, kb, sp):
                        qlo = QS * ps
                        o0 = max(0, 128 * kb - qlo)
                        for h in range(2):
                            if QK8_SCORES:
                                hh = 2 * hp + h
                                t, a = hh // 4, hh % 4
                                nc.tensor.matmul(
                                    sp[:, h, o0:QS],
                                    khT[t][32 * a:32 * a + 32, :,
                                           128 * kb:128 * kb + 128],
                                    qhT[t][32 * a:32 * a + 32, :,
                                           qlo + o0:qlo + QS],
                                    start=True, stop=True, perf_mode=DR,
                                    tile_position=(32 * a, 0))
                            else:
                                nc.tensor.matmul(
                                    sp[:, h, o0:QS],
                                    khT[hp][64 * h:64 * h + 64,
                                            128 * kb:128 * kb + 128],
                                    qhT[hp][64 * h:64 * h + 64,
                                            qlo + o0:qlo + QS],
                                    start=True, stop=True,
                                    tile_position=(64 * h, 0))

                    def attn_span(hp, ps, filler=()):
                        filler = list(filler)
                        qlo = QS * ps
                        nkb = 4 * ps + 4
                        opsum = opsp.tile([DH + 1, 2, QS], F32,
                                          name=f"op{hp}{ps}", tag="op")
                        sp = spsp.tile([128, 2, QS], F32,
                                       name=f"sp{hp}{ps}0", tag="sp")
                        attn_score(hp, ps, 0, sp)
                        stride = max(1, (nkb + len(filler)) // max(1, len(filler))
                                     ) if filler else 0
                        fi = 0
                        for kb in range(nkb):
                            o0 = max(0, 128 * kb - qlo)
                            pt = ptp.tile([128, 2, QS], F16,
                                          name=f"pt{hp}{ps}{kb}", tag="pt")
                            nc.scalar.activation(pt[:, :, o0:QS], sp[:, :, o0:QS],
                                                 EXP, scale=SCALE)
                            # keep the PE fed while ACT computes exp(kb)
                            if kb + 1 < nkb:
                                sp = spsp.tile([128, 2, QS], F32,
                                               name=f"sp{hp}{ps}{kb + 1}",
                                               tag="sp")
                                attn_score(hp, ps, kb + 1, sp)
                            if 128 * kb >= qlo:
                                for h in range(2):
                                    nc.gpsimd.affine_select(
                                        pt[:, h, o0:o0 + 128],
                                        pt[:, h, o0:o0 + 128],
                                        pattern=[[1, 128]],
                                        compare_op=mybir.AluOpType.is_ge,
                                        fill=0.0, base=0, channel_multiplier=-1)
                            for h in range(2):
                                nc.tensor.matmul(
                                    opsum[:, h, o0:QS],
                                    vhh[:, kb, 2 * hp + h, :],
                                    pt[:, h, o0:QS],
                                    start=(kb == 0), stop=(kb == nkb - 1))
                            if filler and fi < len(filler) and \
                                    (kb % stride == stride - 1):
                                filler[fi]()
                                fi += 1
                        for f in filler[fi:]:
                            f()
                        # drain opsum with one cheap copy (frees the PSUM bank
                        # for the next span); normalize off the critical path
                        nd = nrmp.tile([DH + 1, 2, QS], F16,
                                       name=f"nd{hp}{ps}", tag="nd")
                        nc.vector.tensor_copy(nd, opsum[:, :, :])
                        rec = nrmp.tile([1, 2, QS], F32,
                                        name=f"rc{hp}{ps}", tag="rc")
                        nc.vector.reciprocal(rec, nd[DH:DH + 1, :, :])
                        rbc = nrmp.tile([64, 2, QS], F32,
                                        name=f"rb{hp}{ps}", tag="rb")
                        nc.gpsimd.partition_broadcast(rbc, rec)
                        for h in range(2):
                            nc.vector.tensor_mul(
                                cT[hp][64 * h:64 * h + 64, qlo:qlo + QS],
                                nd[0:DH, h, :], rbc[:, h, :])

                    def proj_out_group(sb, dm):
                        fp = auxp.tile([128, 512], F32,
                                       name=f"fp{sb}{dm}", tag="aux")
                        for hp in range(4):
                            nc.tensor.matmul(
                                fp[:, :],
                                cT[hp][:, 128 * sb:128 * sb + 128],
                                wft[:, hp, 512 * dm:512 * dm + 512],
                                start=(hp == 0), stop=(hp == 3))
                        osg = osgp.tile([128, 512], F32,
                                        name=f"os{sb}{dm}", tag="os")
                        nc.vector.tensor_copy(osg, fp[:, :])
                        nc.sync.dma_start(
                            out=out[128 * sb:128 * sb + 128,
                                    512 * dm:512 * dm + 512],
                            in_=osg)

                    # ---------------- emission order ----------------
                    def f_av(sg):
                        return lambda: proj_v(sg)

                    def f_qk(which, pb, sc):
                        return lambda: proj_qk_sc(which, pb, sc)

                    def f_out(sb, dm):
                        return lambda: proj_out_group(sb, dm)

                    proj_qk("q", 0)
                    proj_qk("k", 0)
                    for sg in range(4):
                        proj_v(sg)
                    fill = {
                        (0, 0): [f_av(sg) for sg in range(4, 8)],
                        (0, 1): [f_av(sg) for sg in range(8, 12)],
                        (0, 2): [f_av(sg) for sg in range(12, 16)],
                        (0, 3): [f_qk("q", 1, sc) for sc in range(NSP)]
                              + [f_qk("k", 1, sc) for sc in range(NSP)],
                        (1, 0): [f_qk("q", 2, sc) for sc in range(2)],
                        (1, 1): [f_qk("q", 2, sc) for sc in range(2, 4)],
                        (1, 2): [f_qk("k", 2, sc) for sc in range(2)],
                        (1, 3): [f_qk("k", 2, sc) for sc in range(2, 4)],
                        (2, 0): [f_qk("q", 3, sc) for sc in range(2)],
                        (2, 1): [f_qk("q", 3, sc) for sc in range(2, 4)],
                        (2, 2): [f_qk("k", 3, sc) for sc in range(2)],
                        (2, 3): [f_qk("k", 3, sc) for sc in range(2, 4)],
                        (3, 1): [f_out(sb, dm) for sb in range(0, 4)
                                 for dm in range(2)],
                        (3, 2): [f_out(sb, dm) for sb in range(4, 8)
                                 for dm in range(2)],
                        (3, 3): [f_out(sb, dm) for sb in range(8, 12)
                                 for dm in range(2)],
                    }
                    for hp in range(4):
                        for ps in range(NSP):
                            attn_span(hp, ps, fill.get((hp, ps), ()))
                    for sb in range(12, 16):
                        for dm in range(2):
                            proj_out_group(sb, dm)

            for _rep in range(repeat):
                _phases()
    nc.finalize()
    return nc


def build_in_maps(q, k, v, Wq, bq, Wk, bk, Wv, bv, Wf, bf):
    """Per-core input dicts. Core c = (batch c//2, head-group c%2)."""
    import ml_dtypes
    qk_np = ml_dtypes.float8_e4m3 if QK_FP8 else np.float16

    if QK8_SCORES:
        # col c' = 128*(2t+io) + 32a + pp  <-  head (4t+a), dh = 32io + pp
        perm = np.empty(P, np.int64)
        for t in range(2):
            for io in range(2):
                for a in range(4):
                    pp_ = np.arange(32)
                    perm[128 * (2 * t + io) + 32 * a + pp_] = \
                        (4 * t + a) * 64 + 32 * io + pp_
    else:
        perm = np.arange(P)

    def xt_pack(x, dt=np.float16):  # [S, D] fp32 -> [128, NSP, 8, QS]
        t = np.asarray(x, np.float32).astype(dt)
        # xt[p, sc, db, sj] = x[QS*sc + sj, 128*db + p]
        t = t.reshape(NSP, QS, 8, 128)          # [sc, sj, db, p]
        return np.ascontiguousarray(t.transpose(3, 0, 2, 1))

    def w_pack(w, dt=np.float16):  # [D, P-slice] -> [128, 8, P]
        t = np.asarray(w, np.float32).astype(dt)
        return np.ascontiguousarray(t.reshape(8, 128, -1).transpose(1, 0, 2))

    def wf_pack(w):  # [P-slice(512), D] -> [128, 4, D]
        t = np.asarray(w, np.float32).astype(np.float16)
        return np.ascontiguousarray(t.reshape(4, 128, D).transpose(1, 0, 2))

    vones = np.ones((128, NKB, NH, 1), np.float16)
    bq = np.asarray(bq, np.float32)
    bk = np.asarray(bk, np.float32)
    bv = np.asarray(bv, np.float32)

    in_maps = []
    for c in range(8):
        b, g = c // 2, c % 2
        sl = slice(P * g, P * (g + 1))
        in_maps.append({
            "xq": xt_pack(np.asarray(q)[b], qk_np),
            "xk": xt_pack(np.asarray(k)[b], qk_np),
            "xv": xt_pack(np.asarray(v)[b]),
            "wq": w_pack(np.asarray(Wq)[:, sl][:, perm], qk_np),
            "wk": w_pack(np.asarray(Wk)[:, sl][:, perm], qk_np),
            "wv": w_pack(np.asarray(Wv)[:, sl]),
            "wf": wf_pack(np.asarray(Wf)[sl, :]),
            "bqv": np.ascontiguousarray(bq[sl][perm].reshape(4, 128).T),
            "bkv": np.ascontiguousarray(bk[sl][perm].reshape(4, 128).T),
            "bvv": np.ascontiguousarray(bv[sl].astype(np.float16))[None, :],
            "vones": vones,
        })
    return in_maps


_NC_CACHE = None


def _get_nc():
    global _NC_CACHE
    if _NC_CACHE is None:
        _NC_CACHE = build_core_kernel()
    return _NC_CACHE


def kernel(q, k, v, Wq, bq, Wk, bk, Wv, bv, Wf, bf, trace=False, tmpdir=None):
    in_maps = build_in_maps(q, k, v, Wq, bq, Wk, bk, Wv, bv, Wf, bf)
    bf = np.asarray(bf, np.float32)
    nc = _get_nc()
    kw = {}
    if trace:
        kw = {"trace": True, "tmpdir": tmpdir}
    res = run_bass_kernel_spmd(nc, in_maps, core_ids=list(range(8)), **kw)

    outp = np.empty((4, S, D), np.float32)
    for b in range(4):
        outp[b] = res.results[2 * b]["out"] + res.results[2 * b + 1]["out"] + bf
    if trace:
        return outp, res
    return outp
